# revision 17
# baseline (speedup 1.0000x reference)
"""MoE kernel for Trainium2, expert-parallel across 8 NeuronCores.

Strategy (per the sharding hint "Expert-parallel: shard the experts dim of
W1/W2 across M devices and all-to-all dispatch tokens by expert id"):

- The router (tiny: T x D @ D x E, 0.02% of total FLOPs) runs on host as part
  of the dispatch/sharding step: softmax -> top-2 -> renormalized gates.
- Each of the 8 cores owns one expert. Tokens routed to expert e are gathered,
  padded to a common capacity C, transposed to [D, C] and sent to core e,
  along with that expert's W1/W2 (cast to bf16).
- On-device per core: hT = gelu(W1^T-tiles @ xT), yT = W2-tiles @ hT.
  All matmuls are bf16 x bf16 with fp32 PSUM accumulation; both weight
  matrices stay resident in SBUF; tokens stream in 512-wide chunks.
- Host un-shards: yT -> y, scales by the fp32 gates and scatter-adds into the
  output in expert order (matching the reference's accumulation order).
"""

import os

import ml_dtypes
import numpy as np

import concourse.bass as bass
import concourse.mybir as mybir
import concourse.tile as tile
from concourse import bacc
from concourse.bass_utils import run_bass_kernel_spmd

B, S, D, H, E, TOPK = 4, 2048, 1024, 4096, 8, 2
T = B * S
P = 128
NTOK = 512  # tokens per chunk = matmul moving dim = one PSUM bank of fp32

_BF16 = mybir.dt.bfloat16
_F32 = mybir.dt.float32

# Set by kernel() after each device run so a harness/test can read profiling
# info (exec_time_ns is populated when BASS_TRACE=1).
LAST_RESULTS = None
LAST_C = None


def _chunks_for(C):
    """Split C into chunks of NTOK with one smaller tail chunk (>=128)."""
    sizes = []
    rem = C
    while rem > 0:
        sizes.append(min(NTOK, rem))
        rem -= sizes[-1]
    return sizes


def _build_expert_ffn(d, h, C, repeat=1, psum_bufs=6, bench_no_in=False,
                      bench_no_out=False):
    """Bass program: yT[d, C] = (gelu(x @ W1) @ W2)^T for one expert.

    Inputs (per core):
      xT [d//P, P, C]  bf16   tokens, transposed
      w1 [d//P, P, h]  bf16   up-proj, native [D, H] layout (lhsT tiles)
      w2 [h//P, P, d]  bf16   down-proj, native [H, D] layout (lhsT tiles)
    Output:
      yT [d//P, P, C]  fp32

    repeat>1 re-runs the whole FFN (for benchmarking device time through
    the constant axon transfer overhead).
    """
    dk = d // P  # contraction tiles for up-proj / output tiles for down-proj
    hk = h // P  # output tiles for up-proj / contraction tiles for down-proj
    chunks = _chunks_for(C)
    assert C % P == 0

    nc = bacc.Bacc("TRN2", target_bir_lowering=False, debug=False, num_devices=8)
    xT = nc.dram_tensor("xT", [dk, P, C], _BF16, kind="ExternalInput")
    w1 = nc.dram_tensor("w1", [dk, P, h], _BF16, kind="ExternalInput")
    w2 = nc.dram_tensor("w2", [hk, P, d], _BF16, kind="ExternalInput")
    yT = nc.dram_tensor("yT", [dk, P, C], _F32, kind="ExternalOutput")

    with tile.TileContext(nc) as tc:
        with (
            tc.tile_pool(name="weights", bufs=1) as wpool,
            tc.tile_pool(name="xin", bufs=2) as xpool,
            tc.tile_pool(name="hmid", bufs=1) as hpool,
            tc.tile_pool(name="yout", bufs=1) as opool,
            tc.tile_pool(name="psum", bufs=psum_bufs, space="PSUM") as ppool,
        ):
            w1_sb = wpool.tile([P, dk, h], _BF16)
            w2_sb = wpool.tile([P, hk, d], _BF16)
            for k in range(dk):
                nc.sync.dma_start(w1_sb[:, k, :], w1[k])
            for k in range(hk):
                nc.sync.dma_start(w2_sb[:, k, :], w2[k])

            xT_fixed = None
            if bench_no_in:
                xT_fixed = xpool.tile([P, dk, NTOK], _BF16, tag="xfixed")
                for k in range(dk):
                    nc.sync.dma_start(xT_fixed[:, k, :], xT[k][:, 0:NTOK])

            for _rep in range(repeat):
                off = 0
                for ci, csz in enumerate(chunks):
                    csl = bass.ds(off, csz)
                    off += csz
                    if bench_no_in:
                        xT_sb = xT_fixed
                    else:
                        xT_sb = xpool.tile([P, dk, NTOK], _BF16)
                        for k in range(dk):
                            nc.sync.dma_start(xT_sb[:, k, :csz], xT[k][:, csl])

                    # up-proj + gelu: hT[h, csz] (bf16, on H partitions)
                    hT_sb = hpool.tile([P, hk, NTOK], _BF16)
                    for m in range(hk):
                        ps = ppool.tile([P, NTOK], _F32)
                        for k in range(dk):
                            nc.tensor.matmul(
                                ps[:, :csz],
                                w1_sb[:, k, bass.ts(m, P)],
                                xT_sb[:, k, :csz],
                                start=(k == 0),
                                stop=(k == dk - 1),
                            )
                        nc.scalar.activation(
                            hT_sb[:, m, :csz],
                            ps[:, :csz],
                            mybir.ActivationFunctionType.Gelu,
                        )

                    # down-proj: yT[d, csz] fp32
                    o_sb = opool.tile([P, dk, NTOK], _F32)
                    for m2 in range(dk):
                        ps = ppool.tile([P, NTOK], _F32)
                        for k2 in range(hk):
                            nc.tensor.matmul(
                                ps[:, :csz],
                                w2_sb[:, k2, bass.ts(m2, P)],
                                hT_sb[:, k2, :csz],
                                start=(k2 == 0),
                                stop=(k2 == hk - 1),
                            )
                        nc.vector.tensor_copy(o_sb[:, m2, :csz], ps[:, :csz])
                    if not (bench_no_out and (_rep < repeat - 1 or ci > 0)):
                        nc.sync.dma_start(
                            yT[:, :, csl].rearrange("k p t -> p k t"),
                            o_sb[:, :, :csz],
                        )
    nc.compile()
    return nc


def _router_host(xf, Wr):
    """Softmax -> top-2 (jax.lax.top_k tie-break: lowest index) -> renorm."""
    logits = xf @ Wr  # [T, E] fp32
    m = logits.max(axis=1, keepdims=True)
    ex = np.exp(logits - m)
    probs = ex / ex.sum(axis=1, keepdims=True)
    order = np.argsort(-probs, axis=1, kind="stable")
    idx = order[:, :TOPK]
    w = np.take_along_axis(probs, idx, axis=1)
    w = w / w.sum(axis=1, keepdims=True)
    aux = np.float32(np.sum((probs.mean(axis=0) - np.float32(1.0 / E)) ** 2))
    return probs, idx, w.astype(np.float32), aux


def kernel(x, Wr, W1, W2):
    global LAST_RESULTS, LAST_C
    x = np.asarray(x, dtype=np.float32)
    Wr = np.asarray(Wr, dtype=np.float32)
    W1 = np.asarray(W1, dtype=np.float32)
    W2 = np.asarray(W2, dtype=np.float32)

    xf = np.ascontiguousarray(x.reshape(T, D))
    _, idx, w, aux = _router_host(xf, Wr)

    # Dispatch tokens by expert id (the "all-to-all" of the sharding hint).
    rows_per_e = []
    gates_per_e = []
    for e in range(E):
        sel = (idx == e).any(axis=1)
        rows = np.nonzero(sel)[0]
        slot = np.where(idx[rows, 0] == e, 0, 1)
        g = np.take_along_axis(w[rows], slot[:, None], axis=1)[:, 0]
        rows_per_e.append(rows)
        gates_per_e.append(g)

    cmax = max(len(r) for r in rows_per_e)
    C = max(P, ((cmax + P - 1) // P) * P)
    LAST_C = C

    nc = _build_expert_ffn(D, H, C)

    dk, hk = D // P, H // P
    in_maps = []
    for e in range(E):
        rows = rows_per_e[e]
        xT_np = np.zeros((D, C), dtype=ml_dtypes.bfloat16)
        xT_np[:, : len(rows)] = xf[rows].T.astype(ml_dtypes.bfloat16)
        in_maps.append(
            {
                "xT": xT_np.reshape(dk, P, C),
                "w1": W1[e].astype(ml_dtypes.bfloat16).reshape(dk, P, H),
                "w2": W2[e].astype(ml_dtypes.bfloat16).reshape(hk, P, D),
            }
        )

    # BASS_TRACE=1 routes through an axon NTFF hook that not every container
    # ships (antenv.axon_hooks). Disable tracing rather than crash when the
    # hook module is absent.
    trace_env = {}
    if os.environ.get("BASS_TRACE") and not os.environ.get("BASS_NEVER_TRACE"):
        from concourse._compat import axon_active

        if axon_active():
            try:
                from antenv.axon_hooks import (  # noqa: F401
                    get_axon_ntff_profile_hook,
                )
            except ImportError:
                trace_env["BASS_NEVER_TRACE"] = "1"

    old_env = {k: os.environ.get(k) for k in trace_env}
    os.environ.update(trace_env)
    try:
        res = run_bass_kernel_spmd(nc, in_maps, core_ids=list(range(E)))
    finally:
        for k, v in old_env.items():
            if v is None:
                os.environ.pop(k, None)
            else:
                os.environ[k] = v
    LAST_RESULTS = res

    # Un-shard: gate-scale and scatter-add per expert, in expert order (same
    # fp accumulation order as the reference's dense loop).
    out = np.zeros((T, D), dtype=np.float32)
    for e in range(E):
        rows = rows_per_e[e]
        yT = np.asarray(res.results[e]["yT"], dtype=np.float32).reshape(D, C)
        y = yT[:, : len(rows)].T
        out[rows] += gates_per_e[e][:, None] * y

    return out.reshape(B, S, D), aux
